# revision 40
# baseline (speedup 1.0000x reference)
"""Dynamic depthwise 3x3 conv on 8 Trainium2 cores — 2x-DVE version.

Same sharding/host contract as kernel.py (Phase A). Per tile the 9 taps
split into:
  pass A (dh in {0,1}, 6 taps) — custom PAIR_MAC6 op running in the DVE
      2X_1PORT perf mode (2 bf16 elems/cycle): x is column-interleaved at
      stride 2 (xI2[2u+s] = xpad[r+s, u], built ON-CHIP by the idle ACT
      engine), w packed 6/output, windows of 6 = 3 pairs; a 7-state uop
      program sums each window and writes two outputs as one packed
      (LO,HI) 32-bit store.
  pass B (dh=2, 3 taps) — the 1x segmented-MAC (pages of 3) reading rows
      directly from the resident x block.
  ot = pa + pb (stock bf16 tensor_add, auto-2x).

HBM/core: wA 24 + wB 12 + x 4.3 + y 4 MiB (all bf16).
"""

import sys

sys.path.insert(0, "/opt/trn_rl_repo")

import numpy as np
import ml_dtypes

import concourse.bass as bass
import concourse.bass_isa as bass_isa
import concourse.bacc as bacc
import concourse.tile as tile
from concourse import mybir
from concourse.bass_utils import run_bass_kernel_spmd

import concourse.dve_spec as dve_spec
import concourse.dve_ops as dve_ops
from concourse.dve_spec import AluOp as SAluOp, Spec, Src0, Src1
from concourse.dve_uop import (
    AluInp,
    AluOp,
    DelayInp,
    DveOpSpec,
    InpSel,
    OutPath,
    OutSel,
    Trigger,
    UopConfig,
    ENABLE,
)

from dataclasses import dataclass

# ---------------------------------------------------------------------------
# SEG_MAC (1x, pages of 3) — same op as Phase A / baseline.
# ---------------------------------------------------------------------------

SEG_NAME = "SEG_MAC_ANT"


@dataclass(frozen=True)
class _ResetScan(dve_spec.Scan):
    """scan() that re-seeds from `init` at each SUB_DIM_DONE."""


def _patched_scan_overrides(scans, node_stage):
    seed, step = {}, {}
    for scan in scans:
        d = node_stage[scan]
        init = dve_spec._scan_init(scan)
        seed[d] = dve_spec._node_as_stage(init)
        if isinstance(scan, _ResetScan):
            step[d] = dve_spec._Stage(scan.op, init, scan.expr)
        elif scan._subdim_step is not None:
            step[d] = dve_spec._Stage(
                scan.op, dve_spec.AluInp.CURR_ALU_OUT, scan._subdim_step
            )
    return seed, step


def _segmac_ref(in0, in1, c0, c1, c2):
    return np.cumsum(
        np.asarray(in0, np.float32) * np.asarray(in1, np.float32),
        axis=-1,
        dtype=np.float32,
    )


def get_segmac_op():
    existing = getattr(dve_ops, "_ANT_SEG_MAC", None)
    if existing is not None:
        return existing
    dve_spec._scan_overrides = _patched_scan_overrides
    body = _ResetScan(SAluOp.ADD, Src0 * Src1)
    spec = Spec(body=body, reference=_segmac_ref)
    shas = {}
    for ver in ("v3", "v4"):
        uops = dve_spec.lower(spec, ver=ver)
        shas[ver] = DveOpSpec(name=SEG_NAME, uops=uops, rd1_en=True).sha(ver)
    op = dve_ops.DveOp(SEG_NAME, spec, subdim=True, uops_sha=shas)
    dve_ops.OPS.append(op)
    dve_ops._SUB_OPCODE_FOR_NAME[SEG_NAME] = (
        dve_ops._CUSTOM_DVE_ROW_BASE + len(dve_ops.OPS) - 1
    )
    dve_ops.CUSTOM_DVE_SPECS[SEG_NAME] = spec
    assert dve_ops._SUB_OPCODE_FOR_NAME[SEG_NAME] < 0x20
    dve_ops._ANT_SEG_MAC = op
    return op


# ---------------------------------------------------------------------------
# PAIR_MAC6 (2X_1PORT) — hand-written uop programs.
# ---------------------------------------------------------------------------

PAIR_NAME = "PAIR_MAC6_ANT"
NTAP_A = 6


def _mk_2x_uop(kind: str, nxt: int) -> UopConfig:
    u = UopConfig()
    u.enable_input(InpSel.SRC_0, 0)
    u.enable_input(InpSel.SRC_1, 1)
    u.enable_input(InpSel.SRC_0_HI, 2)
    u.enable_input(InpSel.SRC_1_HI, 3)
    u.require_inp0 = ENABLE
    u.require_inp1 = ENABLE
    # COUNT counts issue cycles; one cycle = one pair at 2x.
    u.repeat_count = 1
    u.trigger = (Trigger.SRC_TENSOR_DONE, Trigger.COUNT, Trigger.NONE)
    u.next_uop = (0, nxt, 0)
    dp = u.datapath_config
    dp[0].enable_alu(AluOp.MULTIPLY, AluInp.PREV_ALU_OUT, AluInp.PREV_DELAY_0)
    dp[0].pass_through_delay(1, 2)
    dp[1].enable_alu(AluOp.MULTIPLY, AluInp.PREV_DELAY_1, AluInp.PREV_DELAY_2)
    dp[1].enable_delay_from_src(DelayInp.PREV_ALU_OUT, 3)
    dp[2].enable_alu(AluOp.ADD, AluInp.PREV_ALU_OUT, AluInp.PREV_DELAY_3)
    if kind == "reset":
        dp[3].enable_alu(AluOp.BYPASS, AluInp.PREV_ALU_OUT)
    else:
        dp[3].enable_alu(AluOp.ADD, AluInp.CURR_ALU_OUT, AluInp.PREV_ALU_OUT)
    dp[4].pass_through_alu()
    if kind == "afinal":
        dp[5].enable_alu(AluOp.BYPASS, AluInp.PREV_ALU_OUT)
    elif kind == "bfinal":
        dp[5].enable_delay_from_src(DelayInp.PREV_ALU_OUT, 4)
        dp[6].enable_alu(AluOp.BYPASS, AluInp.PREV_ALU_OUT)
        dp[6].pass_through_delay(4)
        dp[7].enable_alu(AluOp.BYPASS, AluInp.PREV_ALU_OUT)
        dp[7].pass_through_delay(4)
        u.enable_output(OutSel.ALU_OUT, OutPath.WR0_LO)
        u.enable_output(OutSel.DELAY_4, OutPath.WR0_HI)
    return u


def _mk_1x_uop(kind: str, nxt: int, repeat: int) -> UopConfig:
    u = UopConfig()
    u.enable_input(InpSel.SRC_0, 0)
    u.enable_input(InpSel.SRC_1, 1)
    u.require_inp0 = ENABLE
    u.require_inp1 = ENABLE
    u.repeat_count = repeat
    u.trigger = (Trigger.SRC_TENSOR_DONE, Trigger.COUNT, Trigger.NONE)
    u.next_uop = (0, nxt, 0)
    dp = u.datapath_config
    dp[0].enable_alu(AluOp.MULTIPLY, AluInp.PREV_ALU_OUT, AluInp.PREV_DELAY_0)
    if kind == "reset":
        dp[1].enable_alu(AluOp.BYPASS, AluInp.PREV_ALU_OUT)
    else:
        dp[1].enable_alu(AluOp.ADD, AluInp.CURR_ALU_OUT, AluInp.PREV_ALU_OUT)
    for k in range(2, 8):
        dp[k].pass_through_alu()
    if kind == "final":
        u.enable_output(OutSel.ALU_OUT, OutPath.WR0_LO)
    return u


def get_pair_mac_op():
    existing = getattr(dve_ops, "_ANT_PAIR_MAC6", None)
    if existing is not None:
        return existing
    uops_2x = [
        _mk_2x_uop("reset", 1),
        _mk_2x_uop("acc", 2),
        _mk_2x_uop("afinal", 3),
        _mk_2x_uop("reset", 4),
        _mk_2x_uop("acc", 5),
        _mk_2x_uop("bfinal", 6),
        _mk_2x_uop("reset", 1),
    ]
    uops_1x = [
        _mk_1x_uop("reset", 1, 1),
        _mk_1x_uop("mid", 2, 4),
        _mk_1x_uop("final", 3, 1),
        _mk_1x_uop("reset", 4, 1),
        _mk_1x_uop("mid", 5, 4),
        _mk_1x_uop("final", 6, 1),
        _mk_1x_uop("reset", 1, 1),
    ]
    for u in uops_1x + uops_2x:
        u.validate("v3")

    def _ref(in0, in1, c0, c1, c2):
        p = in0.shape[0]
        a = np.asarray(in0, np.float32).reshape(p, -1, NTAP_A)
        b = np.asarray(in1, np.float32).reshape(p, -1, NTAP_A)
        return (a * b).sum(-1)

    spec = Spec(body=dve_spec.Scan(SAluOp.ADD, Src0 * Src1), reference=_ref)
    op = dve_ops.DveOp(PAIR_NAME, spec, subdim=False, uops_sha={})
    dve_ops.OPS.append(op)
    row = dve_ops._CUSTOM_DVE_ROW_BASE + len(dve_ops.OPS) - 1
    dve_ops._SUB_OPCODE_FOR_NAME[PAIR_NAME] = row
    dve_ops.CUSTOM_DVE_SPECS[PAIR_NAME] = spec
    assert row < 0x20
    dve_ops._COMPILE_CACHE[(PAIR_NAME, "v3")] = DveOpSpec(
        name=PAIR_NAME,
        opcode=row,
        uops=uops_1x,
        uops_2x=uops_2x,
        perf_max=1,
        rd1_en=True,
    )
    dve_ops._ANT_PAIR_MAC6 = op
    return op


PAIR4_NAME = "PAIR_MAC4_ANT"


def get_pair_mac4_op():
    """4-elem windows (2 pairs) -> 1 output; 5-state 2x program. Window
    values {x[c-1], x[c], x[c], x[c+1]} from the stride-2 interleave of one
    row; w = {w0, w1, 0, w2} (c=0 uses {0, 0, w1, w2})."""
    existing = getattr(dve_ops, "_ANT_PAIR_MAC4", None)
    if existing is not None:
        return existing
    uops_2x = [
        _mk_2x_uop("reset", 1),
        _mk_2x_uop("afinal", 2),
        _mk_2x_uop("reset", 3),
        _mk_2x_uop("bfinal", 4),
        _mk_2x_uop("reset", 1),
    ]
    uops_1x = [
        _mk_1x_uop("reset", 1, 1),
        _mk_1x_uop("mid", 2, 1),
        _mk_1x_uop("mid", 3, 1),
        _mk_1x_uop("final", 4, 1),
        _mk_1x_uop("reset", 1, 1),
    ]
    for u in uops_1x + uops_2x:
        u.validate("v3")

    def _ref(in0, in1, c0, c1, c2):
        p = in0.shape[0]
        a = np.asarray(in0, np.float32).reshape(p, -1, 4)
        b = np.asarray(in1, np.float32).reshape(p, -1, 4)
        return (a * b).sum(-1)

    spec = Spec(body=dve_spec.Scan(SAluOp.ADD, Src0 * Src1), reference=_ref)
    op = dve_ops.DveOp(PAIR4_NAME, spec, subdim=False, uops_sha={})
    dve_ops.OPS.append(op)
    row = dve_ops._CUSTOM_DVE_ROW_BASE + len(dve_ops.OPS) - 1
    dve_ops._SUB_OPCODE_FOR_NAME[PAIR4_NAME] = row
    dve_ops.CUSTOM_DVE_SPECS[PAIR4_NAME] = spec
    assert row < 0x20
    dve_ops._COMPILE_CACHE[(PAIR4_NAME, "v3")] = DveOpSpec(
        name=PAIR4_NAME,
        opcode=row,
        uops=uops_1x,
        uops_2x=uops_2x,
        perf_max=1,
        rd1_en=True,
    )
    dve_ops._ANT_PAIR_MAC4 = op
    return op


def emit_custom(nc, op, out_ap, in0_ap, in1_ap, perf_max: int, subdim: bool):
    from concourse.dve_ops import get_dve_sub_opcode

    v = nc.vector
    if op.name not in v.bass.m.ant_custom_dve_ops:
        v.bass.m.ant_custom_dve_ops = sorted(
            {*v.bass.m.ant_custom_dve_ops, op.name}
        )
    shape = (
        bass_isa.CustomDveShape.STT
        if len(in1_ap.shape) > 2
        else bass_isa.CustomDveShape.TTSS
    )
    isa_opcode = v.bass.isa.Opcode[
        f"NEURON_ISA_TPB_OPCODE_CUSTOM_DVE_ANT_{shape.slot()}"
    ].value
    opt = not subdim
    zero = mybir.ImmediateValue(dtype=mybir.dt.float32, value=0.0)
    ins = [
        v.lower_ap(in0_ap, for_isa=True, opt=opt),
        v.lower_ap(in1_ap, for_isa=True, opt=opt),
        zero,
        zero,
    ]
    outs = [v.lower_ap(out_ap, for_isa=True, opt=opt)]
    return v.add_instruction(
        bass_isa.InstCustomDveAnt(
            name=v.bass.get_next_instruction_name(),
            op_name=op.name,
            rd1_en=True,
            subdim=0x02 if subdim else 0,
            imm2=0.0,
            shape=shape,
            row=get_dve_sub_opcode(op.name),
            isa_opcode=isa_opcode,
            ins=ins,
            outs=outs,
            perf_max=perf_max,
        )
    )


def window_ap(sl, dims):
    import bass_rust

    return bass_rust.AP(
        sl.tensor,
        sl.offset,
        [list(sl.ap[0])] + [list(d) for d in dims],
        sl.const_val,
        sl.runtime_checks,
        sl.dep_tracking_offset,
    )


# ---------------------------------------------------------------------------

N, C, H, W = 4, 64, 256, 256
KW = 3
NCORES = 8
HH = H // 2           # rows per core (128)
RB = HH // 2          # rows per partition block (64)
Rh = 8                # rows per h-tile
T = RB // Rh          # h-tiles per core (8)
J = Rh * W            # outputs per tile per partition (2048)
WA = NTAP_A * J       # pass-A w elems per tile (12288)
WB = KW * J           # pass-B w elems per tile (6144)
GX = 1                # front guard of resident x
XRES = GX + (RB + 2) * W + 1   # resident x elems per partition
XI2 = 2 + 2 * J + 6   # interleaved pass-A x buffer
BF16 = mybir.dt.bfloat16
AF = mybir.ActivationFunctionType

_CACHE = {}


def _build():
    seg = get_segmac_op()
    pair = get_pair_mac_op()
    nc = bacc.Bacc("TRN2", target_bir_lowering=False, debug=False, num_devices=NCORES)
    x_in = nc.dram_tensor("x", [128, XRES], BF16, kind="ExternalInput")
    wa_in = nc.dram_tensor("wa", [T, 128, WA], BF16, kind="ExternalInput")
    wb_in = nc.dram_tensor("wb", [T, 128, WB], BF16, kind="ExternalInput")
    y_out = nc.dram_tensor("y", [T, 128, J], BF16, kind="ExternalOutput")

    with tile.TileContext(nc) as tc:
        with (
            tc.tile_pool(name="xr", bufs=1) as xrpool,
            tc.tile_pool(name="xi", bufs=1) as xipool,
            tc.tile_pool(name="wa", bufs=3) as wapool,
            tc.tile_pool(name="wb", bufs=2) as wbpool,
            tc.tile_pool(name="pp", bufs=4) as ppool,
            tc.tile_pool(name="qq", bufs=2) as qpool,
        ):
            xr = xrpool.tile([128, XRES], BF16)
            # chunked resident-x load so tile 0's interleave can start
            # before the whole block lands (chunk k covers 2 tiles' rows)
            nck = 4
            rows_per = (RB + 2 + nck - 1) // nck
            for k in range(nck):
                e0 = GX + k * rows_per * W
                e1 = min(GX + (k + 1) * rows_per * W, XRES)
                if e0 < e1:
                    nc.scalar.dma_start(out=xr[:, e0:e1], in_=x_in[:, e0:e1])
            xis = [
                xipool.tile([128, XI2], BF16, name=f"xi{k}", tag=f"xi{k}")
                for k in range(3)
            ]
            for xi in xis:
                nc.gpsimd.memset(xi[:], 0.0)

            for t in range(T):
                rbase = t * Rh
                wbt = wbpool.tile([128, WB], BF16)
                nc.sync.dma_start(out=wbt[:], in_=wb_in[t])
                wat = wapool.tile([128, WA], BF16)
                for ci in range(2):
                    c0, c1 = ci * (WA // 2), (ci + 1) * (WA // 2)
                    nc.sync.dma_start(out=wat[:, c0:c1], in_=wa_in[t, :, c0:c1])

                # ACT builds xI2 for this tile: xI2[2 + 2*(r*W+u) + s]
                #   = xr[GX + (rbase + r + s)*W + u],  s in {0,1}
                xi = xis[t % 3]
                for s in range(2):
                    nc.scalar.activation(
                        out=window_ap(xi[:, 2 + s:], [[2 * W, Rh], [2, W]]),
                        in_=window_ap(
                            xr[:, GX + (rbase + s) * W:], [[W, Rh], [1, W]]
                        ),
                        func=AF.Copy,
                    )

                # segmac first: it is gated only on wb + resident x, so the
                # DVE starts sooner and the ACT interleave copies get a
                # whole segmac's worth of slack before pair6 needs them.
                pb = qpool.tile([128, J], BF16)
                emit_custom(
                    nc, seg,
                    out_ap=window_ap(pb[:, 0:J], [[1, J], [0, KW]]),
                    in0_ap=window_ap(wbt[:, 0:WB], [[KW, J], [1, KW]]),
                    in1_ap=window_ap(
                        xr[:, GX + (rbase + 2) * W - 1:], [[1, J], [1, KW]]
                    ),
                    perf_max=0, subdim=True,
                )
                pa = ppool.tile([128, J], BF16)
                emit_custom(
                    nc, pair,
                    out_ap=pa[:, 0:J],
                    in0_ap=wat[:, 0:WA],
                    in1_ap=window_ap(xi[:, 0:2 * J], [[2, J], [1, NTAP_A]]),
                    perf_max=1, subdim=False,
                )
                # in-place combine (baseline-proven pattern) — no separate
                # output tile, shorter WAR chains on the Vector queue.
                nc.vector.tensor_add(pa[:], pa[:], pb[:])
                nc.gpsimd.dma_start(out=y_out[t], in_=pa[:])
    nc.compile()
    return nc


def _get_nc():
    if "nc" not in _CACHE:
        _CACHE["nc"] = _build()
    return _CACHE["nc"]


def _pack_core(xh_n: np.ndarray, w5_n: np.ndarray, hf: int):
    """xh_n: [C, H+2, W] H-padded x; w5_n: [C, 9, H, W].
    Returns xres [128, XRES], wa [T, 128, WA], wb [T, 128, WB] (bf16)."""
    xc = xh_n[:, hf * HH:hf * HH + HH + 2, :]          # [C, HH+2, W]
    wc = w5_n[:, :, hf * HH:(hf + 1) * HH, :]          # [C, 9, HH, W]

    # resident x: xres[(hb,c), GX + rho*W + u] = xc[c, hb*RB + rho, u]
    xres = np.zeros((2, C, XRES), dtype=ml_dtypes.bfloat16)
    for hb in range(2):
        blk = xc[:, hb * RB: hb * RB + RB + 2, :].reshape(C, (RB + 2) * W)
        xres[hb, :, GX:GX + (RB + 2) * W] = blk.astype(ml_dtypes.bfloat16)

    # pass-A weights: wa[t,(hb,c), 6*(r*W+u) + 2*dw+dh] = wc[c, dh*3+dw, row, u]
    wcc = wc.reshape(C, KW, KW, 2, T, Rh, W)           # [C, dh, dw, hb, t, r, u]
    waf = wcc[:, 0:2]                                   # dh in {0,1}
    # -> [t, hb, C, r, u, dw, dh]
    wa = np.ascontiguousarray(waf.transpose(4, 3, 0, 5, 6, 2, 1))
    wa[:, :, :, :, 0, 0, :] = 0.0
    wa[:, :, :, :, W - 1, KW - 1, :] = 0.0
    wa = wa.reshape(T, 128, WA).astype(ml_dtypes.bfloat16)

    # pass-B weights: wb[t,(hb,c), 3*(r*W+u) + dw] = wc[c, 6+dw, row, u]
    wbf = wcc[:, 2]                                     # [C, dw, hb, t, r, u]
    wb = np.ascontiguousarray(wbf.transpose(3, 2, 0, 4, 5, 1))
    wb[:, :, :, :, 0, 0] = 0.0
    wb[:, :, :, :, W - 1, KW - 1] = 0.0
    wb = wb.reshape(T, 128, WB).astype(ml_dtypes.bfloat16)
    return xres.reshape(128, XRES), wa, wb


def _make_in_maps(x: np.ndarray, conv_weights: np.ndarray):
    x = np.asarray(x, dtype=np.float32)
    w5 = np.asarray(conv_weights, dtype=np.float32).reshape(N, C, KW * KW, H, W)
    xh = np.pad(x, ((0, 0), (0, 0), (1, 1), (0, 0)))
    in_maps = []
    for i in range(NCORES):
        n, hf = divmod(i, 2)
        xres, wa, wb = _pack_core(xh[n], w5[n], hf)
        in_maps.append({"x": xres, "wa": wa, "wb": wb})
    return in_maps


def kernel(x: np.ndarray, conv_weights: np.ndarray) -> np.ndarray:
    nc = _get_nc()
    in_maps = _make_in_maps(x, conv_weights)
    res = run_bass_kernel_spmd(nc, in_maps, list(range(NCORES)))
    out = np.empty((N, C, H, W), dtype=np.float32)
    for i in range(NCORES):
        n, hf = divmod(i, 2)
        yb = np.asarray(res.results[i]["y"], dtype=np.float32).reshape(
            T, 2, C, Rh, W
        )
        oc = yb.transpose(2, 1, 0, 3, 4).reshape(C, HH, W)
        out[n, :, hf * HH:(hf + 1) * HH, :] = oc
    return out


# revision 42
# speedup vs baseline: 1.0097x; 1.0097x over previous
"""Dynamic depthwise 3x3 conv on 8 Trainium2 cores — 2x-DVE version.

Same sharding/host contract as kernel.py (Phase A). Per tile the 9 taps
split into:
  pass A (dh in {0,1}, 6 taps) — custom PAIR_MAC6 op running in the DVE
      2X_1PORT perf mode (2 bf16 elems/cycle): x is column-interleaved at
      stride 2 (xI2[2u+s] = xpad[r+s, u], built ON-CHIP by the idle ACT
      engine), w packed 6/output, windows of 6 = 3 pairs; a 7-state uop
      program sums each window and writes two outputs as one packed
      (LO,HI) 32-bit store.
  pass B (dh=2, 3 taps) — the 1x segmented-MAC (pages of 3) reading rows
      directly from the resident x block.
  ot = pa + pb (stock bf16 tensor_add, auto-2x).

HBM/core: wA 24 + wB 12 + x 4.3 + y 4 MiB (all bf16).
"""

import sys

sys.path.insert(0, "/opt/trn_rl_repo")

import numpy as np
import ml_dtypes

import concourse.bass as bass
import concourse.bass_isa as bass_isa
import concourse.bacc as bacc
import concourse.tile as tile
from concourse import mybir
from concourse.bass_utils import run_bass_kernel_spmd

import concourse.dve_spec as dve_spec
import concourse.dve_ops as dve_ops
from concourse.dve_spec import AluOp as SAluOp, Spec, Src0, Src1
from concourse.dve_uop import (
    AluInp,
    AluOp,
    DelayInp,
    DveOpSpec,
    InpSel,
    OutPath,
    OutSel,
    Trigger,
    UopConfig,
    ENABLE,
)

from dataclasses import dataclass

# ---------------------------------------------------------------------------
# SEG_MAC (1x, pages of 3) — same op as Phase A / baseline.
# ---------------------------------------------------------------------------

SEG_NAME = "SEG_MAC_ANT"


@dataclass(frozen=True)
class _ResetScan(dve_spec.Scan):
    """scan() that re-seeds from `init` at each SUB_DIM_DONE."""


def _patched_scan_overrides(scans, node_stage):
    seed, step = {}, {}
    for scan in scans:
        d = node_stage[scan]
        init = dve_spec._scan_init(scan)
        seed[d] = dve_spec._node_as_stage(init)
        if isinstance(scan, _ResetScan):
            step[d] = dve_spec._Stage(scan.op, init, scan.expr)
        elif scan._subdim_step is not None:
            step[d] = dve_spec._Stage(
                scan.op, dve_spec.AluInp.CURR_ALU_OUT, scan._subdim_step
            )
    return seed, step


def _segmac_ref(in0, in1, c0, c1, c2):
    return np.cumsum(
        np.asarray(in0, np.float32) * np.asarray(in1, np.float32),
        axis=-1,
        dtype=np.float32,
    )


def get_segmac_op():
    existing = getattr(dve_ops, "_ANT_SEG_MAC", None)
    if existing is not None:
        return existing
    dve_spec._scan_overrides = _patched_scan_overrides
    body = _ResetScan(SAluOp.ADD, Src0 * Src1)
    spec = Spec(body=body, reference=_segmac_ref)
    shas = {}
    for ver in ("v3", "v4"):
        uops = dve_spec.lower(spec, ver=ver)
        shas[ver] = DveOpSpec(name=SEG_NAME, uops=uops, rd1_en=True).sha(ver)
    op = dve_ops.DveOp(SEG_NAME, spec, subdim=True, uops_sha=shas)
    dve_ops.OPS.append(op)
    dve_ops._SUB_OPCODE_FOR_NAME[SEG_NAME] = (
        dve_ops._CUSTOM_DVE_ROW_BASE + len(dve_ops.OPS) - 1
    )
    dve_ops.CUSTOM_DVE_SPECS[SEG_NAME] = spec
    assert dve_ops._SUB_OPCODE_FOR_NAME[SEG_NAME] < 0x20
    dve_ops._ANT_SEG_MAC = op
    return op


# ---------------------------------------------------------------------------
# PAIR_MAC6 (2X_1PORT) — hand-written uop programs.
# ---------------------------------------------------------------------------

PAIR_NAME = "PAIR_MAC6_ANT"
NTAP_A = 6


def _mk_2x_uop(kind: str, nxt: int) -> UopConfig:
    u = UopConfig()
    u.enable_input(InpSel.SRC_0, 0)
    u.enable_input(InpSel.SRC_1, 1)
    u.enable_input(InpSel.SRC_0_HI, 2)
    u.enable_input(InpSel.SRC_1_HI, 3)
    u.require_inp0 = ENABLE
    u.require_inp1 = ENABLE
    # COUNT counts issue cycles; one cycle = one pair at 2x.
    u.repeat_count = 1
    u.trigger = (Trigger.SRC_TENSOR_DONE, Trigger.COUNT, Trigger.NONE)
    u.next_uop = (0, nxt, 0)
    dp = u.datapath_config
    dp[0].enable_alu(AluOp.MULTIPLY, AluInp.PREV_ALU_OUT, AluInp.PREV_DELAY_0)
    dp[0].pass_through_delay(1, 2)
    dp[1].enable_alu(AluOp.MULTIPLY, AluInp.PREV_DELAY_1, AluInp.PREV_DELAY_2)
    dp[1].enable_delay_from_src(DelayInp.PREV_ALU_OUT, 3)
    dp[2].enable_alu(AluOp.ADD, AluInp.PREV_ALU_OUT, AluInp.PREV_DELAY_3)
    if kind == "reset":
        dp[3].enable_alu(AluOp.BYPASS, AluInp.PREV_ALU_OUT)
    else:
        dp[3].enable_alu(AluOp.ADD, AluInp.CURR_ALU_OUT, AluInp.PREV_ALU_OUT)
    dp[4].pass_through_alu()
    if kind == "afinal":
        dp[5].enable_alu(AluOp.BYPASS, AluInp.PREV_ALU_OUT)
    elif kind == "bfinal":
        dp[5].enable_delay_from_src(DelayInp.PREV_ALU_OUT, 4)
        dp[6].enable_alu(AluOp.BYPASS, AluInp.PREV_ALU_OUT)
        dp[6].pass_through_delay(4)
        dp[7].enable_alu(AluOp.BYPASS, AluInp.PREV_ALU_OUT)
        dp[7].pass_through_delay(4)
        u.enable_output(OutSel.ALU_OUT, OutPath.WR0_LO)
        u.enable_output(OutSel.DELAY_4, OutPath.WR0_HI)
    return u


def _mk_1x_uop(kind: str, nxt: int, repeat: int) -> UopConfig:
    u = UopConfig()
    u.enable_input(InpSel.SRC_0, 0)
    u.enable_input(InpSel.SRC_1, 1)
    u.require_inp0 = ENABLE
    u.require_inp1 = ENABLE
    u.repeat_count = repeat
    u.trigger = (Trigger.SRC_TENSOR_DONE, Trigger.COUNT, Trigger.NONE)
    u.next_uop = (0, nxt, 0)
    dp = u.datapath_config
    dp[0].enable_alu(AluOp.MULTIPLY, AluInp.PREV_ALU_OUT, AluInp.PREV_DELAY_0)
    if kind == "reset":
        dp[1].enable_alu(AluOp.BYPASS, AluInp.PREV_ALU_OUT)
    else:
        dp[1].enable_alu(AluOp.ADD, AluInp.CURR_ALU_OUT, AluInp.PREV_ALU_OUT)
    for k in range(2, 8):
        dp[k].pass_through_alu()
    if kind == "final":
        u.enable_output(OutSel.ALU_OUT, OutPath.WR0_LO)
    return u


def get_pair_mac_op():
    existing = getattr(dve_ops, "_ANT_PAIR_MAC6", None)
    if existing is not None:
        return existing
    uops_2x = [
        _mk_2x_uop("reset", 1),
        _mk_2x_uop("acc", 2),
        _mk_2x_uop("afinal", 3),
        _mk_2x_uop("reset", 4),
        _mk_2x_uop("acc", 5),
        _mk_2x_uop("bfinal", 6),
        _mk_2x_uop("reset", 1),
    ]
    uops_1x = [
        _mk_1x_uop("reset", 1, 1),
        _mk_1x_uop("mid", 2, 4),
        _mk_1x_uop("final", 3, 1),
        _mk_1x_uop("reset", 4, 1),
        _mk_1x_uop("mid", 5, 4),
        _mk_1x_uop("final", 6, 1),
        _mk_1x_uop("reset", 1, 1),
    ]
    for u in uops_1x + uops_2x:
        u.validate("v3")

    def _ref(in0, in1, c0, c1, c2):
        p = in0.shape[0]
        a = np.asarray(in0, np.float32).reshape(p, -1, NTAP_A)
        b = np.asarray(in1, np.float32).reshape(p, -1, NTAP_A)
        return (a * b).sum(-1)

    spec = Spec(body=dve_spec.Scan(SAluOp.ADD, Src0 * Src1), reference=_ref)
    op = dve_ops.DveOp(PAIR_NAME, spec, subdim=False, uops_sha={})
    dve_ops.OPS.append(op)
    row = dve_ops._CUSTOM_DVE_ROW_BASE + len(dve_ops.OPS) - 1
    dve_ops._SUB_OPCODE_FOR_NAME[PAIR_NAME] = row
    dve_ops.CUSTOM_DVE_SPECS[PAIR_NAME] = spec
    assert row < 0x20
    dve_ops._COMPILE_CACHE[(PAIR_NAME, "v3")] = DveOpSpec(
        name=PAIR_NAME,
        opcode=row,
        uops=uops_1x,
        uops_2x=uops_2x,
        perf_max=1,
        rd1_en=True,
    )
    dve_ops._ANT_PAIR_MAC6 = op
    return op


PAIR4_NAME = "PAIR_MAC4_ANT"


def get_pair_mac4_op():
    """4-elem windows (2 pairs) -> 1 output; 5-state 2x program. Window
    values {x[c-1], x[c], x[c], x[c+1]} from the stride-2 interleave of one
    row; w = {w0, w1, 0, w2} (c=0 uses {0, 0, w1, w2})."""
    existing = getattr(dve_ops, "_ANT_PAIR_MAC4", None)
    if existing is not None:
        return existing
    uops_2x = [
        _mk_2x_uop("reset", 1),
        _mk_2x_uop("afinal", 2),
        _mk_2x_uop("reset", 3),
        _mk_2x_uop("bfinal", 4),
        _mk_2x_uop("reset", 1),
    ]
    uops_1x = [
        _mk_1x_uop("reset", 1, 1),
        _mk_1x_uop("mid", 2, 1),
        _mk_1x_uop("mid", 3, 1),
        _mk_1x_uop("final", 4, 1),
        _mk_1x_uop("reset", 1, 1),
    ]
    for u in uops_1x + uops_2x:
        u.validate("v3")

    def _ref(in0, in1, c0, c1, c2):
        p = in0.shape[0]
        a = np.asarray(in0, np.float32).reshape(p, -1, 4)
        b = np.asarray(in1, np.float32).reshape(p, -1, 4)
        return (a * b).sum(-1)

    spec = Spec(body=dve_spec.Scan(SAluOp.ADD, Src0 * Src1), reference=_ref)
    op = dve_ops.DveOp(PAIR4_NAME, spec, subdim=False, uops_sha={})
    dve_ops.OPS.append(op)
    row = dve_ops._CUSTOM_DVE_ROW_BASE + len(dve_ops.OPS) - 1
    dve_ops._SUB_OPCODE_FOR_NAME[PAIR4_NAME] = row
    dve_ops.CUSTOM_DVE_SPECS[PAIR4_NAME] = spec
    assert row < 0x20
    dve_ops._COMPILE_CACHE[(PAIR4_NAME, "v3")] = DveOpSpec(
        name=PAIR4_NAME,
        opcode=row,
        uops=uops_1x,
        uops_2x=uops_2x,
        perf_max=1,
        rd1_en=True,
    )
    dve_ops._ANT_PAIR_MAC4 = op
    return op


def emit_custom(nc, op, out_ap, in0_ap, in1_ap, perf_max: int, subdim: bool):
    from concourse.dve_ops import get_dve_sub_opcode

    v = nc.vector
    if op.name not in v.bass.m.ant_custom_dve_ops:
        v.bass.m.ant_custom_dve_ops = sorted(
            {*v.bass.m.ant_custom_dve_ops, op.name}
        )
    shape = (
        bass_isa.CustomDveShape.STT
        if len(in1_ap.shape) > 2
        else bass_isa.CustomDveShape.TTSS
    )
    isa_opcode = v.bass.isa.Opcode[
        f"NEURON_ISA_TPB_OPCODE_CUSTOM_DVE_ANT_{shape.slot()}"
    ].value
    opt = not subdim
    zero = mybir.ImmediateValue(dtype=mybir.dt.float32, value=0.0)
    ins = [
        v.lower_ap(in0_ap, for_isa=True, opt=opt),
        v.lower_ap(in1_ap, for_isa=True, opt=opt),
        zero,
        zero,
    ]
    outs = [v.lower_ap(out_ap, for_isa=True, opt=opt)]
    return v.add_instruction(
        bass_isa.InstCustomDveAnt(
            name=v.bass.get_next_instruction_name(),
            op_name=op.name,
            rd1_en=True,
            subdim=0x02 if subdim else 0,
            imm2=0.0,
            shape=shape,
            row=get_dve_sub_opcode(op.name),
            isa_opcode=isa_opcode,
            ins=ins,
            outs=outs,
            perf_max=perf_max,
        )
    )


def window_ap(sl, dims):
    import bass_rust

    return bass_rust.AP(
        sl.tensor,
        sl.offset,
        [list(sl.ap[0])] + [list(d) for d in dims],
        sl.const_val,
        sl.runtime_checks,
        sl.dep_tracking_offset,
    )


# ---------------------------------------------------------------------------

N, C, H, W = 4, 64, 256, 256
KW = 3
NCORES = 8
HH = H // 2           # rows per core (128)
RB = HH // 2          # rows per partition block (64)
Rh = 8                # rows per h-tile
T = RB // Rh          # h-tiles per core (8)
J = Rh * W            # outputs per tile per partition (2048)
WA = NTAP_A * J       # pass-A w elems per tile (12288)
WB = KW * J           # pass-B w elems per tile (6144)
GX = 1                # front guard of resident x
XRES = GX + (RB + 2) * W + 1   # resident x elems per partition
XI2 = 2 + 2 * J + 6   # interleaved pass-A x buffer
BF16 = mybir.dt.bfloat16
AF = mybir.ActivationFunctionType

_CACHE = {}


def _build():
    seg = get_segmac_op()
    pair = get_pair_mac_op()
    nc = bacc.Bacc("TRN2", target_bir_lowering=False, debug=False, num_devices=NCORES)
    x_in = nc.dram_tensor("x", [128, XRES], BF16, kind="ExternalInput")
    wa_in = nc.dram_tensor("wa", [T, 128, WA], BF16, kind="ExternalInput")
    wb_in = nc.dram_tensor("wb", [T, 128, WB], BF16, kind="ExternalInput")
    y_out = nc.dram_tensor("y", [T, 128, J], BF16, kind="ExternalOutput")

    with tile.TileContext(nc) as tc:
        with (
            tc.tile_pool(name="xr", bufs=1) as xrpool,
            tc.tile_pool(name="xi", bufs=1) as xipool,
            tc.tile_pool(name="wa", bufs=3) as wapool,
            tc.tile_pool(name="wb", bufs=2) as wbpool,
            tc.tile_pool(name="pp", bufs=2) as ppool,
            tc.tile_pool(name="qq", bufs=2) as qpool,
            tc.tile_pool(name="oo", bufs=6) as opool,
        ):
            xr = xrpool.tile([128, XRES], BF16)
            # chunked resident-x load so tile 0's interleave can start
            # before the whole block lands (chunk k covers 2 tiles' rows)
            nck = 4
            rows_per = (RB + 2 + nck - 1) // nck
            for k in range(nck):
                e0 = GX + k * rows_per * W
                e1 = min(GX + (k + 1) * rows_per * W, XRES)
                if e0 < e1:
                    nc.scalar.dma_start(out=xr[:, e0:e1], in_=x_in[:, e0:e1])
            xis = [
                xipool.tile([128, XI2], BF16, name=f"xi{k}", tag=f"xi{k}")
                for k in range(3)
            ]
            for xi in xis:
                nc.gpsimd.memset(xi[:], 0.0)

            for t in range(T):
                rbase = t * Rh
                wbt = wbpool.tile([128, WB], BF16)
                nc.sync.dma_start(out=wbt[:], in_=wb_in[t])
                wat = wapool.tile([128, WA], BF16)
                nc.sync.dma_start(out=wat[:], in_=wa_in[t])

                # ACT builds xI2 for this tile: xI2[2 + 2*(r*W+u) + s]
                #   = xr[GX + (rbase + r + s)*W + u],  s in {0,1}
                xi = xis[t % 3]
                for s in range(2):
                    nc.scalar.activation(
                        out=window_ap(xi[:, 2 + s:], [[2 * W, Rh], [2, W]]),
                        in_=window_ap(
                            xr[:, GX + (rbase + s) * W:], [[W, Rh], [1, W]]
                        ),
                        func=AF.Copy,
                    )

                # segmac first: it is gated only on wb + resident x, so the
                # DVE starts sooner and the ACT interleave copies get a
                # whole segmac's worth of slack before pair6 needs them.
                pb = qpool.tile([128, J], BF16)
                emit_custom(
                    nc, seg,
                    out_ap=window_ap(pb[:, 0:J], [[1, J], [0, KW]]),
                    in0_ap=window_ap(wbt[:, 0:WB], [[KW, J], [1, KW]]),
                    in1_ap=window_ap(
                        xr[:, GX + (rbase + 2) * W - 1:], [[1, J], [1, KW]]
                    ),
                    perf_max=0, subdim=True,
                )
                pa = ppool.tile([128, J], BF16)
                emit_custom(
                    nc, pair,
                    out_ap=pa[:, 0:J],
                    in0_ap=wat[:, 0:WA],
                    in1_ap=window_ap(xi[:, 0:2 * J], [[2, J], [1, NTAP_A]]),
                    perf_max=1, subdim=False,
                )
                ot = opool.tile([128, J], BF16)
                nc.vector.tensor_add(ot[:], pa[:], pb[:])
                nc.gpsimd.dma_start(out=y_out[t], in_=ot[:])
    nc.compile()
    return nc


def _get_nc():
    if "nc" not in _CACHE:
        _CACHE["nc"] = _build()
    return _CACHE["nc"]


def _pack_core(xh_n: np.ndarray, w5_n: np.ndarray, hf: int):
    """xh_n: [C, H+2, W] H-padded x; w5_n: [C, 9, H, W].
    Returns xres [128, XRES], wa [T, 128, WA], wb [T, 128, WB] (bf16)."""
    xc = xh_n[:, hf * HH:hf * HH + HH + 2, :]          # [C, HH+2, W]
    wc = w5_n[:, :, hf * HH:(hf + 1) * HH, :]          # [C, 9, HH, W]

    # resident x: xres[(hb,c), GX + rho*W + u] = xc[c, hb*RB + rho, u]
    xres = np.zeros((2, C, XRES), dtype=ml_dtypes.bfloat16)
    for hb in range(2):
        blk = xc[:, hb * RB: hb * RB + RB + 2, :].reshape(C, (RB + 2) * W)
        xres[hb, :, GX:GX + (RB + 2) * W] = blk.astype(ml_dtypes.bfloat16)

    # pass-A weights: wa[t,(hb,c), 6*(r*W+u) + 2*dw+dh] = wc[c, dh*3+dw, row, u]
    wcc = wc.reshape(C, KW, KW, 2, T, Rh, W)           # [C, dh, dw, hb, t, r, u]
    waf = wcc[:, 0:2]                                   # dh in {0,1}
    # -> [t, hb, C, r, u, dw, dh]
    wa = np.ascontiguousarray(waf.transpose(4, 3, 0, 5, 6, 2, 1))
    wa[:, :, :, :, 0, 0, :] = 0.0
    wa[:, :, :, :, W - 1, KW - 1, :] = 0.0
    wa = wa.reshape(T, 128, WA).astype(ml_dtypes.bfloat16)

    # pass-B weights: wb[t,(hb,c), 3*(r*W+u) + dw] = wc[c, 6+dw, row, u]
    wbf = wcc[:, 2]                                     # [C, dw, hb, t, r, u]
    wb = np.ascontiguousarray(wbf.transpose(3, 2, 0, 4, 5, 1))
    wb[:, :, :, :, 0, 0] = 0.0
    wb[:, :, :, :, W - 1, KW - 1] = 0.0
    wb = wb.reshape(T, 128, WB).astype(ml_dtypes.bfloat16)
    return xres.reshape(128, XRES), wa, wb


def _make_in_maps(x: np.ndarray, conv_weights: np.ndarray):
    x = np.asarray(x, dtype=np.float32)
    w5 = np.asarray(conv_weights, dtype=np.float32).reshape(N, C, KW * KW, H, W)
    xh = np.pad(x, ((0, 0), (0, 0), (1, 1), (0, 0)))
    in_maps = []
    for i in range(NCORES):
        n, hf = divmod(i, 2)
        xres, wa, wb = _pack_core(xh[n], w5[n], hf)
        in_maps.append({"x": xres, "wa": wa, "wb": wb})
    return in_maps


def kernel(x: np.ndarray, conv_weights: np.ndarray) -> np.ndarray:
    nc = _get_nc()
    in_maps = _make_in_maps(x, conv_weights)
    res = run_bass_kernel_spmd(nc, in_maps, list(range(NCORES)))
    out = np.empty((N, C, H, W), dtype=np.float32)
    for i in range(NCORES):
        n, hf = divmod(i, 2)
        yb = np.asarray(res.results[i]["y"], dtype=np.float32).reshape(
            T, 2, C, Rh, W
        )
        oc = yb.transpose(2, 1, 0, 3, 4).reshape(C, HH, W)
        out[n, :, hf * HH:(hf + 1) * HH, :] = oc
    return out
